# revision 22
# baseline (speedup 1.0000x reference)
"""AFM (Attentional Factorization Machine) Trainium2 kernel, v3.

Model (per sample, F=40 fields, K=64 dim, A=64 attn):
  E = V_table[x]                                  [F, K]
  pw[i,j,:] = E[i,:] * E[j,:]                     [F, F, K]
  h = relu(pw @ W1 + b1); s = h @ W2 + b2         [F, F, 1]
  P = softmax_j(s)                                (b2 cancels)
  out = sigmoid(sum_f lin[x] + sum_{i,k} E[i,k] * (P@E)[i,k] * Wf[k] + bf)

Identity: interaction = sum_{i,j} P[i,j] * M[i,j], M[i,j] = pw[i,j] . Wf,
so three TensorE passes over symmetric pair columns (W1, W2, Wf) plus a
wide sample-parallel softmax backend.

v3 over v2: front-end pipelined one block ahead of the matmul phase --
transposes stage into dedicated tail columns of the mv PSUM bank (not
the W1 h-ring), so the PE never waits on the scalar/vector front-end
chain and HAM stays warm; per-og lin matmuls replaced by 4 batched
column-routed matmuls per macro + one fused DVE mask-reduce; warm-up
matmuls ahead of block 0 cover the HAM cold window during the gather
prologue; scps/mvps allocated once (range-level deps handle WAR).

Sharding: pure data-parallel over batch, 256 samples per core.
"""

import sys

import numpy as np

sys.path.insert(0, "/opt/trn_rl_repo")

B, F, K, A, V = 2048, 40, 64, 64, 100000
NCORES = 8
BL = B // NCORES          # 256 samples per core
NMACRO = 2
MS = 128                  # samples per macro
NOG = 64                  # gather groups per macro (2 samples each)
NT = 32                   # transposes per macro (2 og each)
NBLK = 8                  # processing blocks per macro (4 transposes)
NBLK_T = NMACRO * NBLK    # 16 total blocks
NPC = 840                 # stored pair cols per group' (21 diag x 40)
NEV = 11                  # even diagonals d=0,2..20 -> cols 0:440
NOD = 10                  # odd diagonals d=1,3..19  -> cols 440:840
EXT = 60
VROW = 66                 # table row: 64 embed + 1 lin + 1 pad (bf16)
CHUNKS = ((0, 512), (512, 328))
# fraction of relu evictions on ScalarE (rest on DVE); tuned from trace
RELU_SC_NUM = 10
RELU_SC_DEN = 16
# f32 column layout inside the dedicated staging PSUM bank (no matmul
# accumulation groups live there, so start=True zero-region marking by
# transposes / lin matmuls / warm-up matmuls cannot poison accumulators)
STAG0 = 0                 # transpose staging: 8 slots (block parity x j)
SSTR = 40                 # f32 cols between staging slots
LIN0 = 320                # lin-sum regions: 64 f32 cols per macro
DUM0 = 0                  # warm-up scratch (reuses slot region pre-loop)
NWARM = 26                # HAM warm-up matmuls before block 0

_CACHE = {}


def _slot_to_sample():
    s2s = np.full(MS, -1, dtype=np.int64)
    for t in range(NT):
        b, k = t % 4, t // 4
        s2s[32 * b + 4 * k + 0] = 2 * (2 * t)          # sA(og0)
        s2s[32 * b + 4 * k + 1] = 2 * (2 * t + 1)      # sA(og1)
        s2s[32 * b + 4 * k + 2] = 2 * (2 * t) + 1      # sB(og0)
        s2s[32 * b + 4 * k + 3] = 2 * (2 * t + 1) + 1  # sB(og1)
    return s2s


def _og_slots(og):
    # sbuf/psum partition slots for sample A / B of gather group og
    t, o = og // 2, og % 2
    b, kk = t % 4, t // 4
    return 32 * b + 4 * kk + o, 32 * b + 4 * kk + 2 + o


def _build_program(has_bias: bool):
    from concourse import bass, mybir, tile, bacc
    from contextlib import ExitStack

    f32 = mybir.dt.float32
    bf16 = mybir.dt.bfloat16
    i32 = mybir.dt.int32
    AF = mybir.ActivationFunctionType
    ALU = mybir.AluOpType
    AX = mybir.AxisListType

    nc = bacc.Bacc(
        "TRN2", target_bir_lowering=False, debug=False, num_devices=NCORES,
        dynamic_dma_scratch_size=65536,
    )

    xT2_p = nc.dram_tensor("xT2", [2 * F, NMACRO * NOG], i32, kind="ExternalInput").ap()
    va_p = nc.dram_tensor("va", [V, VROW], bf16, kind="ExternalInput").ap()
    ww1_p = nc.dram_tensor("ww1", [128, 128], bf16, kind="ExternalInput").ap()
    w2b_p = nc.dram_tensor("w2b", [128, 512], bf16, kind="ExternalInput").ap()
    wfb_p = nc.dram_tensor("wfb", [128, 512], bf16, kind="ExternalInput").ap()
    linw_p = nc.dram_tensor("linw", [2 * F, 512], bf16, kind="ExternalInput").ap()
    linm_p = nc.dram_tensor("linm", [128, 64], f32, kind="ExternalInput").ap()
    b1v_p = nc.dram_tensor("b1v", [128, 1], f32, kind="ExternalInput").ap()
    bfn_p = nc.dram_tensor("bfn", [128, 1], f32, kind="ExternalInput").ap()
    ident_p = nc.dram_tensor("identb", [128, 128], bf16, kind="ExternalInput").ap()
    out_p = nc.dram_tensor("out", [BL], f32, kind="ExternalOutput").ap()

    def ap_of(t, off, dims):
        # strided view of tile t: free dims = [(stride, n), ...]
        a = t[:]
        return bass.AP(
            a.tensor, a.offset + off, [list(a.ap[0])] + [[s, n] for s, n in dims]
        )

    with tile.TileContext(nc) as tc, ExitStack() as top:
        const = top.enter_context(tc.tile_pool(name="const", bufs=1))

        xT2_sb = const.tile([2 * F, NMACRO * NOG], i32)
        nc.sync.dma_start(out=xT2_sb[:], in_=xT2_p[:])
        ident_sb = const.tile([128, 128], bf16)
        nc.sync.dma_start(out=ident_sb[:], in_=ident_p[:])
        linw_sb = const.tile([2 * F, 512], bf16)
        nc.sync.dma_start(out=linw_sb[:], in_=linw_p[:])
        linm_sb = const.tile([128, 64], f32)
        nc.sync.dma_start(out=linm_sb[:], in_=linm_p[:])
        b1v_sb = const.tile([128, 1], f32)
        nc.sync.dma_start(out=b1v_sb[:], in_=b1v_p[:])
        bfn_sb = const.tile([128, 1], f32)
        nc.sync.dma_start(out=bfn_sb[:], in_=bfn_p[:])
        ww1_sb = const.tile([128, 128], bf16)
        nc.sync.dma_start(out=ww1_sb[:], in_=ww1_p[:])
        w2b_sb = const.tile([128, 512], bf16)
        nc.sync.dma_start(out=w2b_sb[:], in_=w2b_p[:])
        wfb_sb = const.tile([128, 512], bf16)
        nc.sync.dma_start(out=wfb_sb[:], in_=wfb_p[:])

        ga_pool = top.enter_context(tc.tile_pool(name="ga", bufs=1))
        et_pool = top.enter_context(tc.tile_pool(name="et", bufs=3))
        pw_pool = top.enter_context(tc.tile_pool(name="pw", bufs=3))
        rh_pool = top.enter_context(tc.tile_pool(name="rh", bufs=20))
        bk_pool = top.enter_context(tc.tile_pool(name="bk", bufs=2))
        small_pool = top.enter_context(tc.tile_pool(name="small", bufs=4))
        acc_pool = top.enter_context(tc.tile_pool(name="acc", bufs=1, space="PSUM"))
        h_pool = top.enter_context(tc.tile_pool(name="hps", bufs=3, space="PSUM"))
        stag_pool = top.enter_context(tc.tile_pool(name="stag", bufs=1, space="PSUM"))

        # persistent accumulators; macro 1 reuses via range-level WAR deps
        scps = acc_pool.tile([128, 1024], f32, space="PSUM", tag="sc")
        mvps = acc_pool.tile([128, 1024], f32, space="PSUM", tag="mv")
        # one bank for transpose staging + lin sums + warm-up scratch
        stg = stag_pool.tile([128, 512], f32, space="PSUM", tag="stg")

        def stag(bi, j):
            # transpose staging slot: [128, 80] bf16; double-buffered by
            # block parity so a pair's WAR partner is two blocks back
            c0 = STAG0 + SSTR * (4 * (bi % 2) + j)
            return stg[:, c0 : c0 + 40].bitcast(bf16)

        # ---- gathers: one 80-row indirect DMA per og (one idx per
        # partition is the only pattern the Q7 SWDGE honors); the full
        # row [E|lin|pad] lands in one tile, 16 ogs per tile. Emitted
        # lazily so the Pool sequencer stream interleaves with compute.
        # Each completed tile also gets its batched lin matmul: 16 og
        # column sums routed to partition slots via a 0/1 stationary.
        gaR = {}

        def ensure_gather(m, q):
            if (m, q) in gaR:
                return gaR[m, q]
            gr = ga_pool.tile([2 * F, 16 * VROW], bf16, tag=f"gaR{m}{q}")
            for g in range(16):
                og = 16 * q + g
                idx = xT2_sb[:, m * NOG + og : m * NOG + og + 1]
                nc.gpsimd.indirect_dma_start(
                    out=gr[:, g * VROW : (g + 1) * VROW],
                    out_offset=None,
                    in_=va_p[:],
                    in_offset=bass.IndirectOffsetOnAxis(ap=idx, axis=0),
                )
            gaR[m, q] = gr
            return gr

        def lin_pass(ti):
            # batched lin matmul for gather tile ti; emitted in the PE
            # stream right before a transpose pair that waits on strictly
            # later gathers, so it adds zero PE stall by construction
            m, q = ti // 4, ti % 4
            gr = gaR[m, q]
            nc.tensor.matmul(
                out=stg[:, LIN0 + 64 * m + 16 * q : LIN0 + 64 * m + 16 * (q + 1)],
                lhsT=linw_sb[:, 128 * q : 128 * (q + 1)],
                rhs=ap_of(gr, K, [(VROW, 16)]),
                start=True,
                stop=True,
                skip_group_check=True,
            )

        def ensure_block(bi):
            if bi < NBLK_T:
                ensure_gather(bi // NBLK, (bi % NBLK) // 2)

        def tpair(bi, j, et):
            # one transpose pair for block bi (og 2t, 2t+1) + its eviction;
            # interleaved with W2/Wf quadrant groups so each pair waits
            # only on its own two gathers, never convoying ready matmuls
            m, k = bi // NBLK, bi % NBLK
            gr = gaR[m, k // 2]
            t = 4 * k + j
            tp = stag(bi, j)
            lo = 2 * (t % 8)
            nc.tensor.matmul(
                out=tp[0:K, :],
                lhsT=gr[:, lo * VROW : lo * VROW + K],
                rhs=ident_sb[0 : 2 * F, 0 : 2 * F],
                is_transpose=True,
                skip_group_check=True,
                tile_position=(0, 0),
            )
            nc.tensor.matmul(
                out=tp[K:128, :],
                lhsT=gr[:, (lo + 1) * VROW : (lo + 1) * VROW + K],
                rhs=ident_sb[0 : 2 * F, 0 : 2 * F],
                is_transpose=True,
                skip_group_check=True,
                tile_position=(0, K),
            )
            nc.scalar.copy(
                out=ap_of(et, 2 * j * EXT, [(EXT, 2), (1, F)]),
                in_=ap_of(tp, 0, [(F, 2), (1, F)]),
            )

        def frontend_tail(bi, et, e2o):
            # wrap-extend all 8 group blocks: et[g*60+40:g*60+60] = et[g*60:g*60+20]
            nc.scalar.copy(
                out=ap_of(et, F, [(EXT, 8), (1, 20)]),
                in_=ap_of(et, 0, [(EXT, 8), (1, 20)]),
            )
            # e2o[c] = et[c+1] built from core only:
            # e2o[0:39] = et[1:40];  e2o[39:58] = et[0:19]
            nc.scalar.copy(
                out=ap_of(e2o, 0, [(EXT, 8), (1, 39)]),
                in_=ap_of(et, 1, [(EXT, 8), (1, 39)]),
            )
            nc.scalar.copy(
                out=ap_of(e2o, 39, [(EXT, 8), (1, 19)]),
                in_=ap_of(et, 0, [(EXT, 8), (1, 19)]),
            )
            # ---- pair products, in two 4-loc halves (earlier W1 start) ----
            pw = pw_pool.tile([128, 8 * NPC], bf16, tag="pw")
            for h0 in (0, 4):
                nc.vector.tensor_tensor(
                    out=ap_of(pw, h0 * NPC, [(NPC, 4), (F, NEV), (1, F)]),
                    in0=ap_of(et, h0 * EXT, [(EXT, 4), (0, NEV), (1, F)]),
                    in1=ap_of(et, h0 * EXT, [(EXT, 4), (2, NEV), (1, F)]),
                    op=ALU.mult,
                )
                nc.vector.tensor_tensor(
                    out=ap_of(pw, h0 * NPC + 440, [(NPC, 4), (F, NOD), (1, F)]),
                    in0=ap_of(et, h0 * EXT, [(EXT, 4), (0, NOD), (1, F)]),
                    in1=ap_of(e2o, h0 * EXT, [(EXT, 4), (2, NOD), (1, F)]),
                    op=ALU.mult,
                )
            return pw

        # prologue: start gathers, warm the PE (HAM) with dummy matmuls
        ensure_block(0)
        ensure_block(2)
        for _ in range(NWARM):
            nc.tensor.matmul(
                out=stg[:, DUM0 : DUM0 + 128],
                lhsT=ident_sb[:],
                rhs=ident_sb[:],
                start=True,
                stop=True,
                skip_group_check=True,
            )
        et0 = et_pool.tile([128, 8 * EXT], bf16, tag="et")
        e2o0 = et_pool.tile([128, 8 * EXT], bf16, tag="e2o")
        for j in range(4):
            tpair(0, j, et0)
        pw_cur = frontend_tail(0, et0, e2o0)

        relu_ctr = 0
        for bi in range(NBLK_T):
            m, k = bi // NBLK, bi % NBLK
            # next block's gathers ahead of this MM phase; the next
            # frontend is emitted AFTER mm(bi) so the in-order PE queue
            # never blocks ready matmuls behind gather-dependent
            # transposes when the gather stream is the bottleneck
            for d in (1, 2, 3, 4):
                ensure_block(bi + d)
            pw = pw_cur

            # ---- W1 pass: chunk-unit psum ring (one bank per unit) ----
            rhs_tiles = []
            for j in range(4):
                for half in range(2):
                    loc = 2 * j + half
                    rh = rh_pool.tile([128, NPC], bf16, tag="rh")
                    for c0, cn in CHUNKS:
                        hq = h_pool.tile([128, 512], f32, space="PSUM", tag="h")
                        nc.tensor.matmul(
                            out=hq[:, 0:cn],
                            lhsT=ww1_sb[:],
                            rhs=pw[:, loc * NPC + c0 : loc * NPC + c0 + cn],
                            start=True,
                            stop=True,
                        )
                        use_sc = (relu_ctr % 16) < RELU_SC_NUM
                        relu_ctr += 1
                        if has_bias or use_sc:
                            nc.scalar.activation(
                                out=rh[:, c0 : c0 + cn], in_=hq[:, 0:cn],
                                func=AF.Relu, bias=b1v_sb[:],
                            )
                        else:
                            nc.vector.tensor_scalar(
                                out=rh[:, c0 : c0 + cn], in0=hq[:, 0:cn],
                                scalar1=0.0, scalar2=None, op0=ALU.max,
                            )
                    rhs_tiles.append(rh)
            # lin matmul for the tile completed by this block's frontend:
            # its gathers strictly precede tpair(bi+1, 0)'s, so it never
            # adds PE stall here
            if (bi + 1) % 2 == 0 and bi >= 1:
                lin_pass((bi + 1) // 2 - 1)
            # ---- W2/Wf accumulation, interleaved with next block's
            # transpose pairs (each pair gates only on its 2 gathers) ----
            if bi + 1 < NBLK_T:
                et_n = et_pool.tile([128, 8 * EXT], bf16, tag="et")
                e2o_n = et_pool.tile([128, 8 * EXT], bf16, tag="e2o")
            else:
                et_n = e2o_n = None
            # round-robin quadrants so adjacent MMs never share a col
            # group: 4-way PE col-tile concurrency instead of serial runs
            mmq = [[] for _ in range(4)]
            for j in range(4):
                for half in range(2):
                    loc = 2 * j + half
                    b, r = j, 2 * k + half
                    rh = rhs_tiles[loc]
                    for c0, cn in CHUNKS:
                        mmq[b].append((scps, w2b_sb, r, rh, c0, cn, None))
                        mmq[b].append((mvps, wfb_sb, r, pw, c0, cn, loc))
            for c in range(8):
                if et_n is not None and c % 2 == 0:
                    tpair(bi + 1, c // 2, et_n)
                for b in range(4):
                    dst, wsb, r, rhs_t, c0, cn, loc = mmq[b][c]
                    rr = rhs_t[:, loc * NPC + c0 : loc * NPC + c0 + cn] \
                        if loc is not None else rhs_t[:, c0 : c0 + cn]
                    nc.tensor.matmul(
                        out=dst[32 * b : 32 * b + 32, c0 : c0 + cn],
                        lhsT=wsb[:, 32 * r : 32 * r + 32],
                        rhs=rr,
                        tile_position=(0, 32 * b),
                        start=(r == 0),
                        stop=(r == 15),
                        skip_group_check=True,
                    )

            if et_n is not None:
                pw_cur = frontend_tail(bi + 1, et_n, e2o_n)
            if k != NBLK - 1:
                continue
            if m == 1:
                lin_pass(7)  # last tile; all gathers done by now
            # ------------- backend: softmax-weighted reduction -----------
            linsc = small_pool.tile([128, 64], f32, tag="linsc")
            linsum = small_pool.tile([MS, 1], f32, tag="linsum")
            nc.vector.tensor_tensor(
                out=linsc[:],
                in0=stg[:, LIN0 + 64 * m : LIN0 + 64 * m + 64],
                in1=linm_sb[:],
                op=ALU.mult,
            )
            nc.vector.tensor_reduce(
                out=linsum[:], in_=linsc[:], axis=AX.X, op=ALU.add,
            )
            exps = bk_pool.tile([128, NPC], bf16, tag="exps")
            nc.scalar.activation(out=exps[:], in_=scps[:, 0:NPC], func=AF.Exp)
            prods = bk_pool.tile([128, NPC], bf16, tag="prods")
            nc.vector.tensor_tensor(
                out=prods[:], in0=exps[:], in1=mvps[:, 0:NPC], op=ALU.mult
            )
            # direct sums over all 21 stored diagonal blocks
            ap_sum = small_pool.tile([128, F], f32, tag="ap_sum")
            nc.vector.tensor_reduce(
                out=ap_sum[:], in_=ap_of(prods, 0, [(1, F), (F, 21)]),
                axis=AX.X, op=ALU.add, apply_transpose=False,
            )
            ae_sum = small_pool.tile([128, F], f32, tag="ae_sum")
            nc.vector.tensor_reduce(
                out=ae_sum[:], in_=ap_of(exps, 0, [(1, F), (F, 21)]),
                axis=AX.X, op=ALU.add, apply_transpose=False,
            )
            # mirror: doubled tiles [128, 19*80]; odd diags q=0..9 from cols
            # 440+q*40, even diags q'=1..9 from cols q'*40, each block twice
            Dp = bk_pool.tile([128, 19 * 80], bf16, tag="Dp")
            De = bk_pool.tile([128, 19 * 80], bf16, tag="De")
            for srcT, dst, eng in ((prods, Dp, nc.scalar), (exps, De, nc.vector)):
                for off in (0, F):
                    eng.copy(
                        out=ap_of(dst, off, [(80, 10), (1, F)]),
                        in_=ap_of(srcT, 440, [(F, 10), (1, F)]),
                    ) if eng is nc.scalar else eng.tensor_copy(
                        out=ap_of(dst, off, [(80, 10), (1, F)]),
                        in_=ap_of(srcT, 440, [(F, 10), (1, F)]),
                    )
                    eng.copy(
                        out=ap_of(dst, 800 + off, [(80, 9), (1, F)]),
                        in_=ap_of(srcT, F, [(F, 9), (1, F)]),
                    ) if eng is nc.scalar else eng.tensor_copy(
                        out=ap_of(dst, 800 + off, [(80, 9), (1, F)]),
                        in_=ap_of(srcT, F, [(F, 9), (1, F)]),
                    )
            # bp_sum[i] = sum_q D[:, base_q + i]: odd bases 39+78q, even 838+78q'
            bp1 = small_pool.tile([128, F], f32, tag="bp1")
            nc.vector.tensor_reduce(
                out=bp1[:], in_=ap_of(Dp, 39, [(1, F), (78, 10)]),
                axis=AX.X, op=ALU.add, apply_transpose=False,
            )
            bp2 = small_pool.tile([128, F], f32, tag="bp2")
            nc.vector.tensor_reduce(
                out=bp2[:], in_=ap_of(Dp, 838, [(1, F), (78, 9)]),
                axis=AX.X, op=ALU.add, apply_transpose=False,
            )
            be1 = small_pool.tile([128, F], f32, tag="be1")
            nc.vector.tensor_reduce(
                out=be1[:], in_=ap_of(De, 39, [(1, F), (78, 10)]),
                axis=AX.X, op=ALU.add, apply_transpose=False,
            )
            be2 = small_pool.tile([128, F], f32, tag="be2")
            nc.vector.tensor_reduce(
                out=be2[:], in_=ap_of(De, 838, [(1, F), (78, 9)]),
                axis=AX.X, op=ALU.add, apply_transpose=False,
            )
            num = small_pool.tile([128, F], f32, tag="num")
            nc.vector.tensor_tensor(out=num[:], in0=ap_sum[:], in1=bp1[:], op=ALU.add)
            nc.vector.tensor_tensor(out=num[:], in0=num[:], in1=bp2[:], op=ALU.add)
            den = small_pool.tile([128, F], f32, tag="den")
            nc.vector.tensor_tensor(out=den[:], in0=ae_sum[:], in1=be1[:], op=ALU.add)
            nc.vector.tensor_tensor(out=den[:], in0=den[:], in1=be2[:], op=ALU.add)
            rden = small_pool.tile([128, F], f32, tag="rden")
            nc.vector.reciprocal(out=rden[:], in_=den[:])
            c40 = small_pool.tile([128, F], f32, tag="c40")
            nc.vector.tensor_tensor(out=c40[:], in0=num[:], in1=rden[:], op=ALU.mult)
            intr = small_pool.tile([128, 1], f32, tag="intr")
            nc.vector.tensor_reduce(out=intr[:], in_=c40[:], axis=AX.X, op=ALU.add)
            logit = small_pool.tile([128, 1], f32, tag="logit")
            nc.vector.tensor_tensor(
                out=logit[:], in0=intr[:], in1=linsum[:], op=ALU.add
            )
            # sigmoid(x + bf) = 1 / (1 + exp(-x - bf)); bfn holds -bf
            enl = small_pool.tile([128, 1], f32, tag="enl")
            nc.scalar.activation(
                out=enl[:], in_=logit[:], func=AF.Exp, bias=bfn_sb[:], scale=-1.0
            )
            onep = small_pool.tile([128, 1], f32, tag="onep")
            nc.vector.tensor_scalar_add(out=onep[:], in0=enl[:], scalar1=1.0)
            sig = small_pool.tile([128, 1], f32, tag="sig")
            nc.vector.reciprocal(out=sig[:], in_=onep[:])
            nc.sync.dma_start(
                out=out_p[m * MS : (m + 1) * MS].unsqueeze(1), in_=sig[:]
            )

    nc.compile()
    return nc


def _prep_in_maps(x, lin_table, V_table, W1, b1, W2, b2, Wf, bf):
    import ml_dtypes

    # b2 is dropped exactly (softmax shift invariance).
    x = np.asarray(x).astype(np.int32)
    V_table = np.asarray(V_table, dtype=np.float32)
    lin_table = np.asarray(lin_table, dtype=np.float32).reshape(V, 1)
    W1 = np.asarray(W1, dtype=np.float32)
    W2 = np.asarray(W2, dtype=np.float32).reshape(A, 1)
    Wf = np.asarray(Wf, dtype=np.float32).reshape(K, 1)
    b1 = np.asarray(b1, dtype=np.float32).reshape(A)
    bf = np.float32(np.asarray(bf).reshape(-1)[0])
    bff = ml_dtypes.bfloat16

    # augmented bf16 table: [E | lin | pad]
    va = np.zeros((V, VROW), dtype=bff)
    va[:, 0:K] = V_table.astype(bff)
    va[:, K] = lin_table[:, 0].astype(bff)

    ww1 = np.zeros((128, 128), dtype=np.float32)
    ww1[0:K, 0:A] = W1
    ww1[K:128, A:128] = W1
    # 16 stationary variants: variant r has only columns 2r, 2r+1 nonzero
    w2b = np.zeros((128, 512), dtype=np.float32)
    wfb = np.zeros((128, 512), dtype=np.float32)
    for r in range(16):
        w2b[0:A, 32 * r + 2 * r] = W2[:, 0]
        w2b[A:128, 32 * r + 2 * r + 1] = W2[:, 0]
        wfb[0:K, 32 * r + 2 * r] = Wf[:, 0]
        wfb[K:128, 32 * r + 2 * r + 1] = Wf[:, 0]
    # batched lin pass: per gather tile q, stationary column p sums the
    # A-fields (rows 0:40) or B-fields (rows 40:80) iff p is a slot of
    # one of tile q's ogs; the mask then picks each slot's own og column
    linw = np.zeros((2 * F, 512), dtype=np.float32)
    linm = np.zeros((128, 64), dtype=np.float32)
    for q in range(4):
        for g in range(16):
            og = 16 * q + g
            sa, sb = _og_slots(og)
            linw[0:F, 128 * q + sa] = 1.0
            linw[F : 2 * F, 128 * q + sb] = 1.0
            linm[sa, 16 * q + g] = 1.0
            linm[sb, 16 * q + g] = 1.0
    b1v = np.concatenate([b1, b1]).reshape(128, 1).astype(np.float32)
    bfn = np.full((128, 1), -bf, dtype=np.float32)
    ident = np.eye(128, dtype=np.float32)

    xs = x.reshape(NCORES, BL, F)
    in_maps = []
    for c in range(NCORES):
        xc = xs[c]  # [256, 40]
        # xT2[p, m*64+og]: p<40: field p of sample 2og; p>=40: sample 2og+1
        xT2 = np.empty((2 * F, NMACRO * NOG), dtype=np.int32)
        pairs = xc.reshape(NMACRO * NOG, 2, F)
        xT2[0:F, :] = pairs[:, 0, :].T
        xT2[F : 2 * F, :] = pairs[:, 1, :].T
        in_maps.append(
            {
                "xT2": xT2,
                "va": va,
                "ww1": ww1.astype(bff),
                "w2b": w2b.astype(bff),
                "wfb": wfb.astype(bff),
                "linw": linw.astype(bff),
                "linm": linm,
                "b1v": b1v,
                "bfn": bfn,
                "identb": ident.astype(bff),
            }
        )
    return in_maps


def _get_nc(has_bias: bool):
    key = ("nc", has_bias)
    if key not in _CACHE:
        _CACHE[key] = _build_program(has_bias)
    return _CACHE[key]


def kernel(**inputs):
    from concourse.bass_utils import run_bass_kernel_spmd

    has_bias = bool(np.any(np.asarray(inputs["b1"]) != 0))
    nc = _get_nc(has_bias)
    in_maps = _prep_in_maps(**inputs)
    res = run_bass_kernel_spmd(nc, in_maps, core_ids=list(range(NCORES)))
    s2s = _slot_to_sample()
    out = np.empty(B, dtype=np.float32)
    for c in range(NCORES):
        oc = np.asarray(res.results[c]["out"], dtype=np.float32)
        for m in range(NMACRO):
            out[c * BL + m * MS + s2s] = oc[m * MS : (m + 1) * MS]
    return out


# revision 23
# speedup vs baseline: 1.0262x; 1.0262x over previous
"""AFM (Attentional Factorization Machine) Trainium2 kernel, v3.

Model (per sample, F=40 fields, K=64 dim, A=64 attn):
  E = V_table[x]                                  [F, K]
  pw[i,j,:] = E[i,:] * E[j,:]                     [F, F, K]
  h = relu(pw @ W1 + b1); s = h @ W2 + b2         [F, F, 1]
  P = softmax_j(s)                                (b2 cancels)
  out = sigmoid(sum_f lin[x] + sum_{i,k} E[i,k] * (P@E)[i,k] * Wf[k] + bf)

Identity: interaction = sum_{i,j} P[i,j] * M[i,j], M[i,j] = pw[i,j] . Wf,
so three TensorE passes over symmetric pair columns (W1, W2, Wf) plus a
wide sample-parallel softmax backend.

v3 over v2: front-end pipelined one block ahead of the matmul phase --
transposes stage into dedicated tail columns of the mv PSUM bank (not
the W1 h-ring), so the PE never waits on the scalar/vector front-end
chain and HAM stays warm; per-og lin matmuls replaced by 4 batched
column-routed matmuls per macro + one fused DVE mask-reduce; warm-up
matmuls ahead of block 0 cover the HAM cold window during the gather
prologue; scps/mvps allocated once (range-level deps handle WAR).

Sharding: pure data-parallel over batch, 256 samples per core.
"""

import sys

import numpy as np

sys.path.insert(0, "/opt/trn_rl_repo")

B, F, K, A, V = 2048, 40, 64, 64, 100000
NCORES = 8
BL = B // NCORES          # 256 samples per core
NMACRO = 2
MS = 128                  # samples per macro
NOG = 64                  # gather groups per macro (2 samples each)
NT = 32                   # transposes per macro (2 og each)
NBLK = 8                  # processing blocks per macro (4 transposes)
NBLK_T = NMACRO * NBLK    # 16 total blocks
NPC = 840                 # stored pair cols per group' (21 diag x 40)
NEV = 11                  # even diagonals d=0,2..20 -> cols 0:440
NOD = 10                  # odd diagonals d=1,3..19  -> cols 440:840
EXT = 60
VROW = 66                 # table row: 64 embed + 1 lin + 1 pad (bf16)
CHUNKS = ((0, 512), (512, 328))
# fraction of relu evictions on ScalarE (rest on DVE); tuned from trace
RELU_SC_NUM = 10
RELU_SC_DEN = 16
# f32 column layout inside the dedicated staging PSUM bank (no matmul
# accumulation groups live there, so start=True zero-region marking by
# transposes / lin matmuls / warm-up matmuls cannot poison accumulators)
STAG0 = 0                 # transpose staging: 8 slots (block parity x j)
SSTR = 40                 # f32 cols between staging slots
LIN0 = 320                # lin-sum regions: 64 f32 cols per macro
DUM0 = 0                  # warm-up scratch (reuses slot region pre-loop)
NWARM = 26                # HAM warm-up matmuls before block 0

_CACHE = {}


def _slot_to_sample():
    s2s = np.full(MS, -1, dtype=np.int64)
    for t in range(NT):
        b, k = t % 4, t // 4
        s2s[32 * b + 4 * k + 0] = 2 * (2 * t)          # sA(og0)
        s2s[32 * b + 4 * k + 1] = 2 * (2 * t + 1)      # sA(og1)
        s2s[32 * b + 4 * k + 2] = 2 * (2 * t) + 1      # sB(og0)
        s2s[32 * b + 4 * k + 3] = 2 * (2 * t + 1) + 1  # sB(og1)
    return s2s


def _og_slots(og):
    # sbuf/psum partition slots for sample A / B of gather group og
    t, o = og // 2, og % 2
    b, kk = t % 4, t // 4
    return 32 * b + 4 * kk + o, 32 * b + 4 * kk + 2 + o


def _build_program(has_bias: bool):
    from concourse import bass, mybir, tile, bacc
    from contextlib import ExitStack

    f32 = mybir.dt.float32
    bf16 = mybir.dt.bfloat16
    i32 = mybir.dt.int32
    AF = mybir.ActivationFunctionType
    ALU = mybir.AluOpType
    AX = mybir.AxisListType

    nc = bacc.Bacc(
        "TRN2", target_bir_lowering=False, debug=False, num_devices=NCORES,
        dynamic_dma_scratch_size=65536,
    )

    xT2_p = nc.dram_tensor("xT2", [2 * F, NMACRO * NOG], i32, kind="ExternalInput").ap()
    va_p = nc.dram_tensor("va", [V, VROW], bf16, kind="ExternalInput").ap()
    ww1_p = nc.dram_tensor("ww1", [128, 128], bf16, kind="ExternalInput").ap()
    w2b_p = nc.dram_tensor("w2b", [128, 512], bf16, kind="ExternalInput").ap()
    wfb_p = nc.dram_tensor("wfb", [128, 512], bf16, kind="ExternalInput").ap()
    linw_p = nc.dram_tensor("linw", [2 * F, 512], bf16, kind="ExternalInput").ap()
    linm_p = nc.dram_tensor("linm", [128, 64], f32, kind="ExternalInput").ap()
    b1v_p = nc.dram_tensor("b1v", [128, 1], f32, kind="ExternalInput").ap()
    bfn_p = nc.dram_tensor("bfn", [128, 1], f32, kind="ExternalInput").ap()
    ident_p = nc.dram_tensor("identb", [128, 128], bf16, kind="ExternalInput").ap()
    out_p = nc.dram_tensor("out", [BL], f32, kind="ExternalOutput").ap()

    def ap_of(t, off, dims):
        # strided view of tile t: free dims = [(stride, n), ...]
        a = t[:]
        return bass.AP(
            a.tensor, a.offset + off, [list(a.ap[0])] + [[s, n] for s, n in dims]
        )

    with tile.TileContext(nc) as tc, ExitStack() as top:
        const = top.enter_context(tc.tile_pool(name="const", bufs=1))

        xT2_sb = const.tile([2 * F, NMACRO * NOG], i32)
        nc.sync.dma_start(out=xT2_sb[:], in_=xT2_p[:])
        ident_sb = const.tile([128, 128], bf16)
        nc.sync.dma_start(out=ident_sb[:], in_=ident_p[:])
        linw_sb = const.tile([2 * F, 512], bf16)
        nc.sync.dma_start(out=linw_sb[:], in_=linw_p[:])
        linm_sb = const.tile([128, 64], f32)
        nc.sync.dma_start(out=linm_sb[:], in_=linm_p[:])
        b1v_sb = const.tile([128, 1], f32)
        nc.sync.dma_start(out=b1v_sb[:], in_=b1v_p[:])
        bfn_sb = const.tile([128, 1], f32)
        nc.sync.dma_start(out=bfn_sb[:], in_=bfn_p[:])
        ww1_sb = const.tile([128, 128], bf16)
        nc.sync.dma_start(out=ww1_sb[:], in_=ww1_p[:])
        w2b_sb = const.tile([128, 512], bf16)
        nc.sync.dma_start(out=w2b_sb[:], in_=w2b_p[:])
        wfb_sb = const.tile([128, 512], bf16)
        nc.sync.dma_start(out=wfb_sb[:], in_=wfb_p[:])

        ga_pool = top.enter_context(tc.tile_pool(name="ga", bufs=1))
        et_pool = top.enter_context(tc.tile_pool(name="et", bufs=3))
        pw_pool = top.enter_context(tc.tile_pool(name="pw", bufs=3))
        rh_pool = top.enter_context(tc.tile_pool(name="rh", bufs=20))
        bk_pool = top.enter_context(tc.tile_pool(name="bk", bufs=2))
        small_pool = top.enter_context(tc.tile_pool(name="small", bufs=4))
        acc_pool = top.enter_context(tc.tile_pool(name="acc", bufs=1, space="PSUM"))
        h_pool = top.enter_context(tc.tile_pool(name="hps", bufs=3, space="PSUM"))
        stag_pool = top.enter_context(tc.tile_pool(name="stag", bufs=1, space="PSUM"))

        # persistent accumulators; macro 1 reuses via range-level WAR deps
        scps = acc_pool.tile([128, 1024], f32, space="PSUM", tag="sc")
        mvps = acc_pool.tile([128, 1024], f32, space="PSUM", tag="mv")
        # one bank for transpose staging + lin sums + warm-up scratch
        stg = stag_pool.tile([128, 512], f32, space="PSUM", tag="stg")

        def stag(bi, j):
            # transpose staging slot: [128, 80] bf16; double-buffered by
            # block parity so a pair's WAR partner is two blocks back
            c0 = STAG0 + SSTR * (4 * (bi % 2) + j)
            return stg[:, c0 : c0 + 40].bitcast(bf16)

        # ---- gathers: one 80-row indirect DMA per og (one idx per
        # partition is the only pattern the Q7 SWDGE honors); the full
        # row [E|lin|pad] lands in one tile, 16 ogs per tile. Emitted
        # lazily so the Pool sequencer stream interleaves with compute.
        # Each completed tile also gets its batched lin matmul: 16 og
        # column sums routed to partition slots via a 0/1 stationary.
        gaR = {}

        def ensure_gather(m, q):
            if (m, q) in gaR:
                return gaR[m, q]
            gr = ga_pool.tile([2 * F, 16 * VROW], bf16, tag=f"gaR{m}{q}")
            for g in range(16):
                og = 16 * q + g
                idx = xT2_sb[:, m * NOG + og : m * NOG + og + 1]
                nc.gpsimd.indirect_dma_start(
                    out=gr[:, g * VROW : (g + 1) * VROW],
                    out_offset=None,
                    in_=va_p[:],
                    in_offset=bass.IndirectOffsetOnAxis(ap=idx, axis=0),
                )
            gaR[m, q] = gr
            return gr

        def lin_pass(ti):
            # batched lin matmul for gather tile ti; emitted in the PE
            # stream right before a transpose pair that waits on strictly
            # later gathers, so it adds zero PE stall by construction
            m, q = ti // 4, ti % 4
            gr = gaR[m, q]
            nc.tensor.matmul(
                out=stg[:, LIN0 + 64 * m + 16 * q : LIN0 + 64 * m + 16 * (q + 1)],
                lhsT=linw_sb[:, 128 * q : 128 * (q + 1)],
                rhs=ap_of(gr, K, [(VROW, 16)]),
                start=True,
                stop=True,
                skip_group_check=True,
            )

        def ensure_block(bi):
            if bi < NBLK_T:
                ensure_gather(bi // NBLK, (bi % NBLK) // 2)

        def tpair(bi, j, et):
            # one transpose pair for block bi (og 2t, 2t+1) + its eviction;
            # interleaved with W2/Wf quadrant groups so each pair waits
            # only on its own two gathers, never convoying ready matmuls
            m, k = bi // NBLK, bi % NBLK
            gr = gaR[m, k // 2]
            t = 4 * k + j
            tp = stag(bi, j)
            lo = 2 * (t % 8)
            nc.tensor.matmul(
                out=tp[0:K, :],
                lhsT=gr[:, lo * VROW : lo * VROW + K],
                rhs=ident_sb[0 : 2 * F, 0 : 2 * F],
                is_transpose=True,
                skip_group_check=True,
                tile_position=(0, 0),
            )
            nc.tensor.matmul(
                out=tp[K:128, :],
                lhsT=gr[:, (lo + 1) * VROW : (lo + 1) * VROW + K],
                rhs=ident_sb[0 : 2 * F, 0 : 2 * F],
                is_transpose=True,
                skip_group_check=True,
                tile_position=(0, K),
            )
            nc.scalar.copy(
                out=ap_of(et, 2 * j * EXT, [(EXT, 2), (1, F)]),
                in_=ap_of(tp, 0, [(F, 2), (1, F)]),
            )

        def frontend_tail(bi, et, e2o):
            # wrap-extend all 8 group blocks: et[g*60+40:g*60+60] = et[g*60:g*60+20]
            nc.scalar.copy(
                out=ap_of(et, F, [(EXT, 8), (1, 20)]),
                in_=ap_of(et, 0, [(EXT, 8), (1, 20)]),
            )
            # e2o[c] = et[c+1] built from core only:
            # e2o[0:39] = et[1:40];  e2o[39:58] = et[0:19]
            nc.scalar.copy(
                out=ap_of(e2o, 0, [(EXT, 8), (1, 39)]),
                in_=ap_of(et, 1, [(EXT, 8), (1, 39)]),
            )
            nc.scalar.copy(
                out=ap_of(e2o, 39, [(EXT, 8), (1, 19)]),
                in_=ap_of(et, 0, [(EXT, 8), (1, 19)]),
            )
            # ---- pair products, in two 4-loc halves (earlier W1 start) ----
            pw = pw_pool.tile([128, 8 * NPC], bf16, tag="pw")
            for h0 in (0, 4):
                nc.vector.tensor_tensor(
                    out=ap_of(pw, h0 * NPC, [(NPC, 4), (F, NEV), (1, F)]),
                    in0=ap_of(et, h0 * EXT, [(EXT, 4), (0, NEV), (1, F)]),
                    in1=ap_of(et, h0 * EXT, [(EXT, 4), (2, NEV), (1, F)]),
                    op=ALU.mult,
                )
                nc.vector.tensor_tensor(
                    out=ap_of(pw, h0 * NPC + 440, [(NPC, 4), (F, NOD), (1, F)]),
                    in0=ap_of(et, h0 * EXT, [(EXT, 4), (0, NOD), (1, F)]),
                    in1=ap_of(e2o, h0 * EXT, [(EXT, 4), (2, NOD), (1, F)]),
                    op=ALU.mult,
                )
            return pw

        # prologue: start gathers, warm the PE (HAM) with dummy matmuls
        ensure_block(0)
        ensure_block(2)
        for _ in range(NWARM):
            nc.tensor.matmul(
                out=stg[:, DUM0 : DUM0 + 128],
                lhsT=ident_sb[:],
                rhs=ident_sb[:],
                start=True,
                stop=True,
                skip_group_check=True,
            )
        et0 = et_pool.tile([128, 8 * EXT], bf16, tag="et")
        e2o0 = et_pool.tile([128, 8 * EXT], bf16, tag="e2o")
        for j in range(4):
            tpair(0, j, et0)
        pw_cur = frontend_tail(0, et0, e2o0)

        relu_ctr = 0
        for bi in range(NBLK_T):
            m, k = bi // NBLK, bi % NBLK
            # next block's gathers ahead of this MM phase; the next
            # frontend is emitted AFTER mm(bi) so the in-order PE queue
            # never blocks ready matmuls behind gather-dependent
            # transposes when the gather stream is the bottleneck
            for d in (1, 2, 3, 4):
                ensure_block(bi + d)
            pw = pw_cur

            # ---- W1 pass: chunk-unit psum ring (one bank per unit) ----
            rhs_tiles = []
            for j in range(4):
                for half in range(2):
                    loc = 2 * j + half
                    rh = rh_pool.tile([128, NPC], bf16, tag="rh")
                    for c0, cn in CHUNKS:
                        hq = h_pool.tile([128, 512], f32, space="PSUM", tag="h")
                        nc.tensor.matmul(
                            out=hq[:, 0:cn],
                            lhsT=ww1_sb[:],
                            rhs=pw[:, loc * NPC + c0 : loc * NPC + c0 + cn],
                            start=True,
                            stop=True,
                        )
                        use_sc = (relu_ctr % 16) < RELU_SC_NUM
                        relu_ctr += 1
                        if has_bias or use_sc:
                            nc.scalar.activation(
                                out=rh[:, c0 : c0 + cn], in_=hq[:, 0:cn],
                                func=AF.Relu, bias=b1v_sb[:],
                            )
                        else:
                            nc.vector.tensor_scalar(
                                out=rh[:, c0 : c0 + cn], in0=hq[:, 0:cn],
                                scalar1=0.0, scalar2=None, op0=ALU.max,
                            )
                    rhs_tiles.append(rh)
            # lin matmul for the tile completed by this block's frontend:
            # its gathers strictly precede tpair(bi+1, 0)'s, so it never
            # adds PE stall here
            if (bi + 1) % 2 == 0 and bi >= 1:
                lin_pass((bi + 1) // 2 - 1)
            # ---- W2/Wf accumulation, interleaved with next block's
            # transpose pairs (each pair gates only on its 2 gathers) ----
            if bi + 1 < NBLK_T:
                et_n = et_pool.tile([128, 8 * EXT], bf16, tag="et")
                e2o_n = et_pool.tile([128, 8 * EXT], bf16, tag="e2o")
            else:
                et_n = e2o_n = None
            for j in range(4):
                if et_n is not None:
                    tpair(bi + 1, j, et_n)
                for half in range(2):
                    loc = 2 * j + half
                    b, r = j, 2 * k + half
                    rh = rhs_tiles[loc]
                    for c0, cn in CHUNKS:
                        nc.tensor.matmul(
                            out=scps[32 * b : 32 * b + 32, c0 : c0 + cn],
                            lhsT=w2b_sb[:, 32 * r : 32 * r + 32],
                            rhs=rh[:, c0 : c0 + cn],
                            tile_position=(0, 32 * b),
                            start=(r == 0),
                            stop=(r == 15),
                            skip_group_check=True,
                        )
                        nc.tensor.matmul(
                            out=mvps[32 * b : 32 * b + 32, c0 : c0 + cn],
                            lhsT=wfb_sb[:, 32 * r : 32 * r + 32],
                            rhs=pw[:, loc * NPC + c0 : loc * NPC + c0 + cn],
                            tile_position=(0, 32 * b),
                            start=(r == 0),
                            stop=(r == 15),
                            skip_group_check=True,
                        )

            if et_n is not None:
                pw_cur = frontend_tail(bi + 1, et_n, e2o_n)
            if k != NBLK - 1:
                continue
            if m == 1:
                lin_pass(7)  # last tile; all gathers done by now
            # ------------- backend: softmax-weighted reduction -----------
            linsc = small_pool.tile([128, 64], f32, tag="linsc")
            linsum = small_pool.tile([MS, 1], f32, tag="linsum")
            nc.vector.tensor_tensor(
                out=linsc[:],
                in0=stg[:, LIN0 + 64 * m : LIN0 + 64 * m + 64],
                in1=linm_sb[:],
                op=ALU.mult,
            )
            nc.vector.tensor_reduce(
                out=linsum[:], in_=linsc[:], axis=AX.X, op=ALU.add,
            )
            exps = bk_pool.tile([128, NPC], bf16, tag="exps")
            nc.scalar.activation(out=exps[:], in_=scps[:, 0:NPC], func=AF.Exp)
            prods = bk_pool.tile([128, NPC], bf16, tag="prods")
            nc.vector.tensor_tensor(
                out=prods[:], in0=exps[:], in1=mvps[:, 0:NPC], op=ALU.mult
            )
            # direct sums over all 21 stored diagonal blocks
            ap_sum = small_pool.tile([128, F], f32, tag="ap_sum")
            nc.vector.tensor_reduce(
                out=ap_sum[:], in_=ap_of(prods, 0, [(1, F), (F, 21)]),
                axis=AX.X, op=ALU.add, apply_transpose=False,
            )
            ae_sum = small_pool.tile([128, F], f32, tag="ae_sum")
            nc.vector.tensor_reduce(
                out=ae_sum[:], in_=ap_of(exps, 0, [(1, F), (F, 21)]),
                axis=AX.X, op=ALU.add, apply_transpose=False,
            )
            # mirror: doubled tiles [128, 19*80]; odd diags q=0..9 from cols
            # 440+q*40, even diags q'=1..9 from cols q'*40, each block twice
            Dp = bk_pool.tile([128, 19 * 80], bf16, tag="Dp")
            De = bk_pool.tile([128, 19 * 80], bf16, tag="De")
            for srcT, dst, on_sc in ((prods, Dp, True), (exps, De, False)):
                for off in (0, F):
                    if on_sc:
                        nc.scalar.copy(
                            out=ap_of(dst, off, [(80, 10), (1, F)]),
                            in_=ap_of(srcT, 440, [(F, 10), (1, F)]),
                        )
                        nc.scalar.copy(
                            out=ap_of(dst, 800 + off, [(80, 9), (1, F)]),
                            in_=ap_of(srcT, F, [(F, 9), (1, F)]),
                        )
                    else:
                        nc.vector.tensor_copy(
                            out=ap_of(dst, off, [(80, 10), (1, F)]),
                            in_=ap_of(srcT, 440, [(F, 10), (1, F)]),
                        )
                        nc.vector.tensor_copy(
                            out=ap_of(dst, 800 + off, [(80, 9), (1, F)]),
                            in_=ap_of(srcT, F, [(F, 9), (1, F)]),
                        )
            # bp_sum[i] = sum_q D[:, base_q + i]: odd bases 39+78q, even 838+78q'
            bp1 = small_pool.tile([128, F], f32, tag="bp1")
            nc.vector.tensor_reduce(
                out=bp1[:], in_=ap_of(Dp, 39, [(1, F), (78, 10)]),
                axis=AX.X, op=ALU.add, apply_transpose=False,
            )
            bp2 = small_pool.tile([128, F], f32, tag="bp2")
            nc.vector.tensor_reduce(
                out=bp2[:], in_=ap_of(Dp, 838, [(1, F), (78, 9)]),
                axis=AX.X, op=ALU.add, apply_transpose=False,
            )
            be1 = small_pool.tile([128, F], f32, tag="be1")
            nc.vector.tensor_reduce(
                out=be1[:], in_=ap_of(De, 39, [(1, F), (78, 10)]),
                axis=AX.X, op=ALU.add, apply_transpose=False,
            )
            be2 = small_pool.tile([128, F], f32, tag="be2")
            nc.vector.tensor_reduce(
                out=be2[:], in_=ap_of(De, 838, [(1, F), (78, 9)]),
                axis=AX.X, op=ALU.add, apply_transpose=False,
            )
            num = small_pool.tile([128, F], f32, tag="num")
            nc.vector.tensor_tensor(out=num[:], in0=ap_sum[:], in1=bp1[:], op=ALU.add)
            nc.vector.tensor_tensor(out=num[:], in0=num[:], in1=bp2[:], op=ALU.add)
            den = small_pool.tile([128, F], f32, tag="den")
            nc.vector.tensor_tensor(out=den[:], in0=ae_sum[:], in1=be1[:], op=ALU.add)
            nc.vector.tensor_tensor(out=den[:], in0=den[:], in1=be2[:], op=ALU.add)
            rden = small_pool.tile([128, F], f32, tag="rden")
            nc.vector.reciprocal(out=rden[:], in_=den[:])
            c40 = small_pool.tile([128, F], f32, tag="c40")
            nc.vector.tensor_tensor(out=c40[:], in0=num[:], in1=rden[:], op=ALU.mult)
            intr = small_pool.tile([128, 1], f32, tag="intr")
            nc.vector.tensor_reduce(out=intr[:], in_=c40[:], axis=AX.X, op=ALU.add)
            logit = small_pool.tile([128, 1], f32, tag="logit")
            nc.vector.tensor_tensor(
                out=logit[:], in0=intr[:], in1=linsum[:], op=ALU.add
            )
            # sigmoid(x + bf) = 1 / (1 + exp(-x - bf)); bfn holds -bf
            enl = small_pool.tile([128, 1], f32, tag="enl")
            nc.scalar.activation(
                out=enl[:], in_=logit[:], func=AF.Exp, bias=bfn_sb[:], scale=-1.0
            )
            onep = small_pool.tile([128, 1], f32, tag="onep")
            nc.vector.tensor_scalar_add(out=onep[:], in0=enl[:], scalar1=1.0)
            sig = small_pool.tile([128, 1], f32, tag="sig")
            nc.vector.reciprocal(out=sig[:], in_=onep[:])
            nc.sync.dma_start(
                out=out_p[m * MS : (m + 1) * MS].unsqueeze(1), in_=sig[:]
            )

    nc.compile()
    return nc


def _prep_in_maps(x, lin_table, V_table, W1, b1, W2, b2, Wf, bf):
    import ml_dtypes

    # b2 is dropped exactly (softmax shift invariance).
    x = np.asarray(x).astype(np.int32)
    V_table = np.asarray(V_table, dtype=np.float32)
    lin_table = np.asarray(lin_table, dtype=np.float32).reshape(V, 1)
    W1 = np.asarray(W1, dtype=np.float32)
    W2 = np.asarray(W2, dtype=np.float32).reshape(A, 1)
    Wf = np.asarray(Wf, dtype=np.float32).reshape(K, 1)
    b1 = np.asarray(b1, dtype=np.float32).reshape(A)
    bf = np.float32(np.asarray(bf).reshape(-1)[0])
    bff = ml_dtypes.bfloat16

    # augmented bf16 table: [E | lin | pad]
    va = np.zeros((V, VROW), dtype=bff)
    va[:, 0:K] = V_table.astype(bff)
    va[:, K] = lin_table[:, 0].astype(bff)

    ww1 = np.zeros((128, 128), dtype=np.float32)
    ww1[0:K, 0:A] = W1
    ww1[K:128, A:128] = W1
    # 16 stationary variants: variant r has only columns 2r, 2r+1 nonzero
    w2b = np.zeros((128, 512), dtype=np.float32)
    wfb = np.zeros((128, 512), dtype=np.float32)
    for r in range(16):
        w2b[0:A, 32 * r + 2 * r] = W2[:, 0]
        w2b[A:128, 32 * r + 2 * r + 1] = W2[:, 0]
        wfb[0:K, 32 * r + 2 * r] = Wf[:, 0]
        wfb[K:128, 32 * r + 2 * r + 1] = Wf[:, 0]
    # batched lin pass: per gather tile q, stationary column p sums the
    # A-fields (rows 0:40) or B-fields (rows 40:80) iff p is a slot of
    # one of tile q's ogs; the mask then picks each slot's own og column
    linw = np.zeros((2 * F, 512), dtype=np.float32)
    linm = np.zeros((128, 64), dtype=np.float32)
    for q in range(4):
        for g in range(16):
            og = 16 * q + g
            sa, sb = _og_slots(og)
            linw[0:F, 128 * q + sa] = 1.0
            linw[F : 2 * F, 128 * q + sb] = 1.0
            linm[sa, 16 * q + g] = 1.0
            linm[sb, 16 * q + g] = 1.0
    b1v = np.concatenate([b1, b1]).reshape(128, 1).astype(np.float32)
    bfn = np.full((128, 1), -bf, dtype=np.float32)
    ident = np.eye(128, dtype=np.float32)

    xs = x.reshape(NCORES, BL, F)
    in_maps = []
    for c in range(NCORES):
        xc = xs[c]  # [256, 40]
        # xT2[p, m*64+og]: p<40: field p of sample 2og; p>=40: sample 2og+1
        xT2 = np.empty((2 * F, NMACRO * NOG), dtype=np.int32)
        pairs = xc.reshape(NMACRO * NOG, 2, F)
        xT2[0:F, :] = pairs[:, 0, :].T
        xT2[F : 2 * F, :] = pairs[:, 1, :].T
        in_maps.append(
            {
                "xT2": xT2,
                "va": va,
                "ww1": ww1.astype(bff),
                "w2b": w2b.astype(bff),
                "wfb": wfb.astype(bff),
                "linw": linw.astype(bff),
                "linm": linm,
                "b1v": b1v,
                "bfn": bfn,
                "identb": ident.astype(bff),
            }
        )
    return in_maps


def _get_nc(has_bias: bool):
    key = ("nc", has_bias)
    if key not in _CACHE:
        _CACHE[key] = _build_program(has_bias)
    return _CACHE[key]


def kernel(**inputs):
    from concourse.bass_utils import run_bass_kernel_spmd

    has_bias = bool(np.any(np.asarray(inputs["b1"]) != 0))
    nc = _get_nc(has_bias)
    in_maps = _prep_in_maps(**inputs)
    res = run_bass_kernel_spmd(nc, in_maps, core_ids=list(range(NCORES)))
    s2s = _slot_to_sample()
    out = np.empty(B, dtype=np.float32)
    for c in range(NCORES):
        oc = np.asarray(res.results[c]["out"], dtype=np.float32)
        for m in range(NMACRO):
            out[c * BL + m * MS + s2s] = oc[m * MS : (m + 1) * MS]
    return out
